# revision 15
# baseline (speedup 1.0000x reference)
"""Trainium2 Bass kernel for NeuroVPR Vanilla SNN (3-layer LIF, T=3).

Data-parallel over batch: B=16384 -> 2048 per core x 8 cores.

Math (per timestep, per layer): v = (v_prev + h)/2; s = (v>=1); v *= (1-s).
The LIF recurrence is homogeneous and the decay is a power of 2, so each
layer runs in a scaled basis u_t = 2^t * 2c * v_t (c = weight prescale,
32 for L1 / 16 for L2-L3, lifting weights out of fp8's subnormal range):
    u_t = m_{t-1} + 2^t * psum_t      (the *0.5 decay cancels)
    s_t = (u_t >= 2^t * 2c)
    m_t = u_t * (u_t < 2^t * 2c)
The 2^t factor rides the ScalarE extract's free `scale` field; thresholds
double each timestep (exact powers of 2). Spike decisions match the fp32
recurrence up to matmul quantization error.

All matmuls are fp8e4 perf_mode=DoubleRow (K=256/instr, warm issue rate
216 ns at N=512). Hidden-layer spikes live in a +/-1 (ScalarE Sign, t=1)
or +/-0.5 (VectorE is_ge/sub, t=0 and t=2) basis: the next layer's
ScalarE extract scale absorbs the basis change (SSC) and the row-sum
correction rides that layer's bias column, precomputed on host from the
quantized weights. L1's bias rides a ones row appended to x. L3 at t=2
needs no extract or correction matmul: u = 0.125*m3 + psum on VectorE
and the correction folds into a per-partition threshold column
(tensor_scalar with AP scalar).

Schedule is COLUMN-MAJOR: the batch splits into four 512-column blocks
and each block runs t=0,1,2 consecutively (columns are independent LIF
chains, so blocks are independent). Each phase = one L1 [2816x512] GEMM
(22 DR matmuls, ~4.75us); the previous phase's LIF + L2 + L3 chains run
on Scalar/Vector DURING the next phase, with their matmuls hooked into
its k-loop. This spreads the elementwise work evenly (~75% V / ~70% S
per phase) instead of stacking the whole t=2 dependency cone after the
last L1 matmul (the old layer-major tail was ~16us of serialized
VectorE; the HAM throttle also halves PE rate after ~68-75us of
sustained activity, so the tail must avoid PE). Only the last block's
short t=2 chain (~4us) trails the final matmul.

x tiles ([128, 2048] fp8 per (t, k-slab, half-batch)) stream in
consumption order ~one phase ahead, issues split across the sync and
scalar HW DGE queues (the only HW queues; gpsimd's software queue is
slow and its epilogue drains are expensive when used). w1's k=0 slab
rides the sync queue first: it gates the first matmul.
"""
import os
import numpy as np
import ml_dtypes

B, T, D = 16384, 3, 2752
DP = 2816          # D padded to 11*256
KD = DP // 256     # 11 DoubleRow contraction slabs
H, O = 256, 100
OP = 112           # O padded so the DoubleRow pair-stride is 16B-aligned
NCORES = 8
BC = B // NCORES   # 2048
NB = 512           # matmul free-dim block = column-block width
HB = BC // 2       # half-batch per x tile (1024)
NBLK = 4           # column blocks per core

SC1, SC2 = 32.0, 8.0    # weight prescale: L1; L2/L3 (+/-1 spike basis)
TH1, TH2 = 64.0, 32.0   # base thresholds (scaled x2 each timestep)
EPS = 0.0625            # tie-break so Sign(u - (th-EPS)) == +/-1 with s=1 at u==th

_compiled = None
last_results = None  # BassKernelResults of the most recent run (for profiling)


def _build():
    from contextlib import ExitStack
    import concourse.bass as bass
    import concourse.mybir as mybir
    import concourse.tile as tile
    from concourse import bacc

    f8 = mybir.dt.float8e4
    bf16 = mybir.dt.bfloat16
    f32 = mybir.dt.float32
    A = mybir.AluOpType
    DR = mybir.MatmulPerfMode.DoubleRow
    IDENT = mybir.ActivationFunctionType.Identity
    SIGN = mybir.ActivationFunctionType.Sign

    nc = bacc.Bacc("TRN2", target_bir_lowering=False, debug=False)
    x = nc.dram_tensor("x", [T, KD, 2, 128, HB * 2], f8, kind="ExternalInput").ap()
    w1 = nc.dram_tensor("w1", [128, KD * 2 * H], f8, kind="ExternalInput").ap()
    w2 = nc.dram_tensor("w2", [128, 2 * H], f8, kind="ExternalInput").ap()
    w3 = nc.dram_tensor("w3", [128, 2 * OP], f8, kind="ExternalInput").ap()
    bias = nc.dram_tensor("bias", [128, 22], f32, kind="ExternalInput").ap()
    out = nc.dram_tensor("out", [O, BC], f32, kind="ExternalOutput").ap()

    with tile.TileContext(nc) as tc, ExitStack() as ctx:
        wp = ctx.enter_context(tc.tile_pool(name="wp", bufs=1))
        xp = ctx.enter_context(tc.tile_pool(name="xp", bufs=44))
        pp1 = ctx.enter_context(tc.tile_pool(name="pp1", bufs=2, space="PSUM"))
        pp23 = ctx.enter_context(tc.tile_pool(name="pp23", bufs=2, space="PSUM"))
        sp = ctx.enter_context(tc.tile_pool(name="sp", bufs=1))
        tp = ctx.enter_context(tc.tile_pool(name="tp", bufs=6))

        # ---- ACT warmup first: fully host-data-independent ----
        wu = wp.tile([128, 8], bf16)
        wub = wp.tile([128, 1], f32)
        nc.vector.memset(wu[:, :], 0.0)
        nc.vector.memset(wub[:, :], 0.0)
        nc.scalar.activation(wu[:, 0:4], wu[:, 4:8], IDENT, bias=wub[:, 0:1])

        # ---- weights / bias tiles (DMAs issued in the initial-prefetch
        # interleave below, in deadline order against the early x tiles) ----
        w1t = wp.tile([128, KD * 2 * H], f8)
        w1o = w1t[:, :].rearrange("p (k two m) -> p k two m", k=KD, two=2)
        bt = wp.tile([128, 22], f32)
        w2t = wp.tile([128, 2 * H], f8)
        w2o = w2t[:, :].rearrange("p (two m) -> p two m", two=2)
        w3t = wp.tile([128, 2 * OP], f8)
        w3o = w3t[:, :].rearrange("p (two m) -> p two m", two=2)
        # column layout (host fills): 0-5 beta1[t,h]; 6-11 beta2[t,h];
        # 12-14 beta3[t]; 15-17 -(2^t*TH1-EPS); 18-20 -(2^t*TH2-EPS);
        # 21 final L3 threshold 16 - rs3/2 - SC2*b3
        B1 = lambda t, h: bt[:, 2 * t + h: 2 * t + h + 1]
        B2 = lambda t, h: bt[:, 6 + 2 * t + h: 6 + 2 * t + h + 1]
        B3 = lambda t: bt[:, 12 + t: 13 + t]
        N1 = lambda t: bt[:, 15 + t: 16 + t]
        N2 = lambda t: bt[:, 18 + t: 19 + t]
        TH3C = bt[:, 21:22]

        # ---- persistent state (m = scaled membrane, written at t=0) ----
        m1 = [sp.tile([128, BC], bf16, tag=f"m1_{h}", name=f"m1_{h}")
              for h in range(2)]
        m2 = [sp.tile([128, BC], bf16, tag=f"m2_{h}", name=f"m2_{h}")
              for h in range(2)]
        m3 = sp.tile([128, BC], bf16, tag="m3")
        s1 = sp.tile([128, 2 * BC], f8, tag="s1")
        s2 = sp.tile([128, 2 * BC], f8, tag="s2")
        s1r = s1[:, :].rearrange("p (two n) -> p two n", two=2)
        s2r = s2[:, :].rearrange("p (two n) -> p two n", two=2)
        outsb = sp.tile([128, BC], f32, tag="outsb")

        xt = {}  # (t, k, half) -> x tile handle [128, 2*HB]

        def x_fetch(t, k, half, q=None):
            xt[t, k, half] = xp.tile([128, 2 * HB], f8, tag="x",
                                     name=f"x_{t}_{k}_{half}")
            (q or nc.sync).dma_start(out=xt[t, k, half][:, :],
                                     in_=x[t, k, half, :, :])

        SSC = [2.0, 2.0, 4.0]  # L2/L3 extract scale: 2^t x (2 if +/-0.5 basis)

        def lif(ps, m_ap, s_ap, bcol, nthcol, th, t, P=128, sc=None):
            """Scaled-LIF on one [P, NB] psum span (t < T-1): ScalarE
            extract + bf16 VectorE ops; spike on ScalarE Sign at t=1,
            VectorE is_ge/sub at t=0."""
            hb = tp.tile([128, NB], bf16, tag="hb", name="hb")[:P, :]
            nc.scalar.activation(hb, ps, IDENT, bias=bcol[:P, :],
                                 scale=float(2 ** t) if sc is None else sc)
            if t == 0:
                u = hb
            else:
                u = tp.tile([128, NB], bf16, tag="u", name="u")[:P, :]
                nc.vector.tensor_tensor(u, m_ap, hb, A.add)
            if s_ap is not None:
                if t == 1:
                    nc.scalar.activation(s_ap, u, SIGN, bias=nthcol[:P, :])
                else:
                    nc.vector.tensor_scalar(s_ap, u, th * 2 ** t, 0.5,
                                            A.is_ge, A.subtract)
            nc.vector.scalar_tensor_tensor(m_ap, u, th * 2 ** t, u,
                                           A.is_lt, A.mult)

        def l1_lif(blk, ps):
            """L1 LIF at t=T-1 for one block, straight from PSUM:
            u = 0.25*m1 + psum; s1 = Sign(u - (TH1-EPS)) (+/-1 basis)."""
            cs = slice(blk * NB, (blk + 1) * NB)
            for h in range(2):
                u = tp.tile([128, NB], bf16, tag="u", name="u")
                nc.vector.scalar_tensor_tensor(u, m1[h][:, cs], 0.25,
                                               ps[:, h * NB:(h + 1) * NB],
                                               A.mult, A.add)
                nc.scalar.activation(s1[:, h * BC + blk * NB:
                                        h * BC + (blk + 1) * NB],
                                     u, SIGN, bias=N1(0))

        def l1_phase(t, blk, hooks=None):
            """One [2816 x 512] L1 GEMM phase: k inner, 2 MMs per slab into
            the two h-banks of one [128,1024] psum tile, then this
            (t, blk)'s L1 LIF. `hooks[k]` interleaves previous chains'
            matmuls and x prefetches into the PE stream."""
            half, b = blk // 2, blk % 2
            cs = slice(blk * NB, (blk + 1) * NB)
            ps = pp1.tile([128, 2 * NB], f32, tag="ps1", name=f"ps_{t}_{blk}")
            for k in range(KD):
                for fn in (hooks or {}).get(k, []):
                    fn()
                xr = xt[t, k, half][:, :].rearrange("p (two n) -> p two n",
                                                    two=2)
                for h in range(2):
                    nc.tensor.matmul(
                        ps[:, h * NB:(h + 1) * NB],
                        w1o[:, k, :, h * 128:(h + 1) * 128],
                        xr[:, :, b * NB:(b + 1) * NB],
                        start=(k == 0), stop=(k == KD - 1), perf_mode=DR,
                        skip_group_check=True)
            if t < T - 1:
                for h in range(2):
                    lif(ps[:, h * NB:(h + 1) * NB], m1[h][:, cs],
                        s1[:, h * BC + blk * NB: h * BC + (blk + 1) * NB],
                        B1(t, h), N1(t), TH1, t)
            else:
                l1_lif(blk, ps)

        cht = {}  # (t, blk) -> shared L2/L3 chain psum tile

        def l2_blk(t, blk):
            """L2 chain for one (t, block): 2 MMs into one ps23 tile's
            h-banks, then per-h LIF (t<2) or the t=2 endgame extract."""
            cs = slice(blk * NB, (blk + 1) * NB)
            ch = pp23.tile([128, 2 * NB], f32, tag="ps23", name=f"ch_{t}_{blk}")
            cht[t, blk] = ch
            for h in range(2):
                nc.tensor.matmul(ch[:, h * NB:(h + 1) * NB],
                                 w2o[:, :, h * 128:(h + 1) * 128],
                                 s1r[:, :, blk * NB:(blk + 1) * NB],
                                 start=True, stop=True, perf_mode=DR,
                                 skip_group_check=True)
            for h in range(2):
                if t < T - 1:
                    lif(ch[:, h * NB:(h + 1) * NB], m2[h][:, cs],
                        s2[:, h * BC + blk * NB: h * BC + (blk + 1) * NB],
                        B2(t, h), N2(t), TH2, t, sc=SSC[t])
                else:
                    hb = tp.tile([128, NB], bf16, tag="hb", name="hb")
                    nc.scalar.activation(hb, ch[:, h * NB:(h + 1) * NB],
                                         IDENT, bias=B2(2, h), scale=SSC[2])
                    u = tp.tile([128, NB], bf16, tag="u", name="u")
                    nc.vector.tensor_tensor(u, m2[h][:, cs], hb, A.add)
                    nc.vector.tensor_scalar(
                        s2[:, h * BC + blk * NB: h * BC + (blk + 1) * NB],
                        u, TH2 * 4, 0.5, A.is_ge, A.subtract)

        def l3_blk(t, blk):
            """L3 chain for one (t, block), reusing the L2 chain tile's
            first bank. t<2 updates m3; t=2 compares u against the
            per-partition threshold column and DMAs the output block."""
            cs = slice(blk * NB, (blk + 1) * NB)
            ch = cht[t, blk]
            nc.tensor.matmul(ch[:OP, 0:NB], w3o[:, :, :],
                             s2r[:, :, blk * NB:(blk + 1) * NB],
                             start=True, stop=True, perf_mode=DR,
                             skip_group_check=True)
            if t < T - 1:
                lif(ch[:OP, 0:NB], m3[:OP, cs], None, B3(t), None, TH2, t,
                    P=OP, sc=SSC[t])
            else:
                u = tp.tile([128, NB], bf16, tag="u", name="u")[:OP, :]
                nc.vector.scalar_tensor_tensor(u, m3[:OP, cs], 0.125,
                                               ch[:OP, 0:NB], A.mult, A.add)
                nc.vector.tensor_scalar(outsb[:OP, cs], u, TH3C[:OP, :],
                                        None, A.is_ge)
                nc.scalar.dma_start(out=out[:, cs], in_=outsb[:O, cs])

        # ---- phase sequence and hook schedule ----
        # Initial prefetch covers t=0 AND t=1 of half0 (the first two
        # phases), interleaved across the sync/scalar queues in deadline
        # order so every streamed tile later has >= one phase of slack.
        # Remaining x tiles stream in consumption order, 6 per phase.
        fetch_list = [(2, k, 0) for k in range(KD)] \
            + [(0, k, 1) for k in range(KD)] \
            + [(1, k, 1) for k in range(KD)] \
            + [(2, k, 1) for k in range(KD)]

        # sync: w1 k0 slab (gates the first matmul), then even x(0) tiles.
        # scalar: w1 k1-3 (needed by MM k1), odd x(0) tiles, remaining w1
        # and small tensors interleaved by deadline, then x(1) odd tiles.
        nc.sync.dma_start(out=w1t[:, 0:512], in_=w1[:, 0:512])
        nc.scalar.dma_start(out=w1t[:, 512:2048], in_=w1[:, 512:2048])
        for k in (0, 1, 2, 3):
            x_fetch(0, k, 0, nc.sync if k % 2 == 0 else nc.scalar)
        nc.scalar.dma_start(out=w1t[:, 2048:4096], in_=w1[:, 2048:4096])
        for k in (4, 5):
            x_fetch(0, k, 0, nc.sync if k % 2 == 0 else nc.scalar)
        nc.scalar.dma_start(out=w1t[:, 4096:KD * 512], in_=w1[:, 4096:KD * 512])
        for k in (6, 7, 8, 9, 10):
            x_fetch(0, k, 0, nc.sync if k % 2 == 0 else nc.scalar)
        nc.scalar.dma_start(out=bt[:, :], in_=bias[:, :])
        for k in range(KD):
            x_fetch(1, k, 0, nc.sync if k % 2 == 1 else nc.scalar)
        nc.scalar.dma_start(out=w2t[:, :], in_=w2[:, :])
        nc.scalar.dma_start(out=w3t[:, :], in_=w3[:, :])

        fi = 0
        phases = [(blk, t) for blk in range(NBLK) for t in range(T)]
        for pi, (blk, t) in enumerate(phases):
            hooks = {}
            # stream the next ~phase of x tiles (skip during phase 0: its
            # queues are still busy with the initial prefetch)
            if pi >= 1:
                n = min(6, len(fetch_list) - fi)
                for j in range(n):
                    tt_, kk, hh = fetch_list[fi + j]
                    hooks.setdefault(2 * j, []).append(
                        lambda tt_=tt_, kk=kk, hh=hh: x_fetch(tt_, kk, hh))
                fi += n
            # previous phase's L2 at k5, its L3 at k10 (L3 needs s2 from
            # the L2 chain, ~2us later)
            if pi >= 1:
                pb, pt = phases[pi - 1][0], phases[pi - 1][1]
                hooks.setdefault(5, []).append(
                    lambda pt=pt, pb=pb: l2_blk(pt, pb))
                hooks.setdefault(10, []).append(
                    lambda pt=pt, pb=pb: l3_blk(pt, pb))
            l1_phase(t, blk, hooks)
        # trailing chain of the last block's t=2
        l2_blk(2, NBLK - 1)
        l3_blk(2, NBLK - 1)

    nc.compile()
    return nc


def kernel(dvs, W1, b1, W2, b2, W3, b3):
    global _compiled, last_results
    from concourse.bass_utils import run_bass_kernel_spmd

    if _compiled is None:
        _compiled = _build()
    nc = _compiled

    f8 = ml_dtypes.float8_e4m3

    def q8(a, scale):
        return np.clip(a * scale, -240.0, 240.0).astype(f8)

    # x: [B, T, D] -> fp8 [T, KD, 128, 2, B]  (d = k*256 + two*128 + p)
    x8 = q8(dvs, 1.0).transpose(1, 2, 0)          # [T, D, B]
    X = np.zeros((T, KD, 2, 128, B), dtype=f8)
    X.reshape(T, DP, B)[:, :D, :] = x8
    X.reshape(T, DP, B)[:, D, :] = f8(1.0)        # bias row (w1 row D = c1*b1)
    X = np.ascontiguousarray(X.transpose(0, 1, 3, 2, 4))  # [T, KD, 128, 2, B]

    # w1: [DP, H] scaled by SC1 -> [128, KD, 2, H]
    w1p = np.zeros((KD, 2, 128, H), dtype=f8)
    w1p.reshape(DP, H)[:D, :] = q8(W1.T, SC1)
    w1p.reshape(DP, H)[D, :] = q8(b1, SC1)
    w1p = np.ascontiguousarray(w1p.transpose(2, 0, 1, 3)).reshape(128, KD * 2 * H)
    # w2/w3 scaled by SC2 (+/-1 spike basis)
    w2q = q8(W2.T, SC2)                            # [H, H] j-major
    w2p = np.ascontiguousarray(
        w2q.reshape(2, 128, H).transpose(1, 0, 2)).reshape(128, 2 * H)
    w3q = np.zeros((H, OP), dtype=f8)
    w3q[:, :O] = q8(W3.T, SC2)
    w3p = np.ascontiguousarray(
        w3q.reshape(2, 128, OP).transpose(1, 0, 2)).reshape(128, 2 * OP)

    # bias/threshold columns; row-sum corrections use the quantized weights
    rs2 = w2q.astype(np.float64).sum(axis=0)       # [H]
    rs3 = w3q.astype(np.float64).sum(axis=0)       # [OP]
    bc = np.zeros((128, 22), dtype=np.float32)
    for t in range(T):
        p2 = float(2 ** t)
        for h in range(2):
            bc[:, 6 + 2 * t + h] = p2 * (rs2[h * 128:(h + 1) * 128]
                                         + 2 * SC2 * b2[h * 128:(h + 1) * 128])
        bc[:OP, 12 + t] = p2 * rs3
        bc[:O, 12 + t] += p2 * 2 * SC2 * b3
        bc[:, 15 + t] = -(p2 * TH1 - EPS)
        bc[:, 18 + t] = -(p2 * TH2 - EPS)
    # final L3 threshold column: s3 = (0.125*m3 + psum >= 16 - corr),
    # corr = rs3/2 + SC2*b3 (the +/-0.5 s2 basis row-sum correction)
    bc[:OP, 21] = 16.0 - rs3 / 2
    bc[:O, 21] -= SC2 * b3

    in_maps = []
    for c in range(NCORES):
        xc = X[:, :, :, :, c * BC:(c + 1) * BC]    # [T, KD, 128, 2, BC]
        xc = np.ascontiguousarray(
            xc.reshape(T, KD, 128, 2, 2, HB).transpose(0, 1, 4, 2, 3, 5)
        ).reshape(T, KD, 2, 128, 2 * HB)           # [T, KD, half, 128, 2*HB]
        in_maps.append({"x": xc, "w1": w1p, "w2": w2p, "w3": w3p, "bias": bc})

    trace = bool(os.environ.get("SNN_TRACE"))
    last_results = run_bass_kernel_spmd(nc, in_maps, core_ids=list(range(NCORES)),
                                        trace=trace)
    outp = np.empty((B, O), dtype=np.float32)
    for c in range(NCORES):
        outp[c * BC:(c + 1) * BC, :] = last_results.results[c]["out"].T
    return outp


# revision 21
# speedup vs baseline: 1.0346x; 1.0346x over previous
"""Trainium2 Bass kernel for NeuroVPR Vanilla SNN (3-layer LIF, T=3).

Data-parallel over batch: B=16384 -> 2048 per core x 8 cores.

Math (per timestep, per layer): v = (v_prev + h)/2; s = (v>=1); v *= (1-s).
The LIF recurrence is homogeneous and the decay is a power of 2, so each
layer runs in a scaled basis u_t = 2^t * 2c * v_t (c = weight prescale,
32 for L1 / 16 for L2-L3, lifting weights out of fp8's subnormal range):
    u_t = m_{t-1} + 2^t * psum_t      (the *0.5 decay cancels)
    s_t = (u_t >= 2^t * 2c)
    m_t = u_t * (u_t < 2^t * 2c)
The 2^t factor rides the ScalarE extract's free `scale` field; thresholds
double each timestep (exact powers of 2). Spike decisions match the fp32
recurrence up to matmul quantization error.

All matmuls are fp8e4 perf_mode=DoubleRow (K=256/instr, warm issue rate
216 ns at N=512). Hidden-layer spikes live in a +/-1 (ScalarE Sign, t=1)
or +/-0.5 (VectorE is_ge/sub, t=0 and t=2) basis: the next layer's
ScalarE extract scale absorbs the basis change (SSC) and the row-sum
correction rides that layer's bias column, precomputed on host from the
quantized weights. L1's bias rides a ones row appended to x. L3 at t=2
needs no extract or correction matmul: u = 0.125*m3 + psum on VectorE
and the correction folds into a per-partition threshold column
(tensor_scalar with AP scalar).

Schedule is COLUMN-MAJOR: the batch splits into four 512-column blocks
and each block runs t=0,1,2 consecutively (columns are independent LIF
chains, so blocks are independent). Each phase = one L1 [2816x512] GEMM
(22 DR matmuls, ~4.75us); the previous phase's LIF + L2 + L3 chains run
on Scalar/Vector DURING the next phase, with their matmuls hooked into
its k-loop. This spreads the elementwise work evenly (~75% V / ~70% S
per phase) instead of stacking the whole t=2 dependency cone after the
last L1 matmul (the old layer-major tail was ~16us of serialized
VectorE; the HAM throttle also halves PE rate after ~68-75us of
sustained activity, so the tail must avoid PE). Only the last block's
short t=2 chain (~4us) trails the final matmul.

x tiles ([128, 2048] fp8 per (t, k-slab, half-batch)) stream in
consumption order ~one phase ahead, issues split across the sync and
scalar HW DGE queues (the only HW queues; gpsimd's software queue is
slow and its epilogue drains are expensive when used). w1's k=0 slab
rides the sync queue first: it gates the first matmul.
"""
import os
import numpy as np
import ml_dtypes

B, T, D = 16384, 3, 2752
DP = 2816          # D padded to 11*256
KD = DP // 256     # 11 DoubleRow contraction slabs
H, O = 256, 100
OP = 112           # O padded so the DoubleRow pair-stride is 16B-aligned
NCORES = 8
BC = B // NCORES   # 2048
NB = 512           # matmul free-dim block = column-block width
HB = BC // 2       # half-batch per x tile (1024)
NBLK = 4           # column blocks per core

SC1, SC2 = 32.0, 8.0    # weight prescale: L1; L2/L3 (+/-1 spike basis)
TH1, TH2 = 64.0, 32.0   # base thresholds (scaled x2 each timestep)
EPS = 0.0625            # tie-break so Sign(u - (th-EPS)) == +/-1 with s=1 at u==th

_compiled = None
last_results = None  # BassKernelResults of the most recent run (for profiling)


def _build():
    from contextlib import ExitStack
    import concourse.bass as bass
    import concourse.mybir as mybir
    import concourse.tile as tile
    from concourse import bacc

    f8 = mybir.dt.float8e4
    bf16 = mybir.dt.bfloat16
    f32 = mybir.dt.float32
    A = mybir.AluOpType
    DR = mybir.MatmulPerfMode.DoubleRow
    IDENT = mybir.ActivationFunctionType.Identity
    SIGN = mybir.ActivationFunctionType.Sign

    nc = bacc.Bacc("TRN2", target_bir_lowering=False, debug=False)
    # x packed per (t, column-block): one contiguous [128, KD*2*512] slab
    # (k-major per partition line) so each phase streams exactly the bytes
    # it consumes, in consumption order.
    x = nc.dram_tensor("x", [T, NBLK, 128, KD * 2 * NB], f8,
                       kind="ExternalInput").ap()
    w1 = nc.dram_tensor("w1", [128, KD * 2 * H], f8, kind="ExternalInput").ap()
    w2 = nc.dram_tensor("w2", [128, 2 * H], f8, kind="ExternalInput").ap()
    w3 = nc.dram_tensor("w3", [128, 2 * OP], f8, kind="ExternalInput").ap()
    bias = nc.dram_tensor("bias", [128, 22], f32, kind="ExternalInput").ap()
    out = nc.dram_tensor("out", [O, BC], f32, kind="ExternalOutput").ap()

    with tile.TileContext(nc) as tc, ExitStack() as ctx:
        wp = ctx.enter_context(tc.tile_pool(name="wp", bufs=1))
        xp = ctx.enter_context(tc.tile_pool(name="xp", bufs=4))
        pp1 = ctx.enter_context(tc.tile_pool(name="pp1", bufs=2, space="PSUM"))
        pp23 = ctx.enter_context(tc.tile_pool(name="pp23", bufs=2, space="PSUM"))
        sp = ctx.enter_context(tc.tile_pool(name="sp", bufs=1))
        tp = ctx.enter_context(tc.tile_pool(name="tp", bufs=6))

        # ---- ACT warmup first: fully host-data-independent ----
        wu = wp.tile([128, 8], bf16)
        wub = wp.tile([128, 1], f32)
        nc.vector.memset(wu[:, :], 0.0)
        nc.vector.memset(wub[:, :], 0.0)
        nc.scalar.activation(wu[:, 0:4], wu[:, 4:8], IDENT, bias=wub[:, 0:1])

        # ---- weights / bias tiles (DMAs issued in the initial-prefetch
        # interleave below, in deadline order against the early x tiles) ----
        w1t = wp.tile([128, KD * 2 * H], f8)
        w1o = w1t[:, :].rearrange("p (k two m) -> p k two m", k=KD, two=2)
        bt = wp.tile([128, 22], f32)
        w2t = wp.tile([128, 2 * H], f8)
        w2o = w2t[:, :].rearrange("p (two m) -> p two m", two=2)
        w3t = wp.tile([128, 2 * OP], f8)
        w3o = w3t[:, :].rearrange("p (two m) -> p two m", two=2)
        # column layout (host fills): 0-5 beta1[t,h]; 6-11 beta2[t,h];
        # 12-14 beta3[t]; 15-17 -(2^t*TH1-EPS); 18-20 -(2^t*TH2-EPS);
        # 21 final L3 threshold 16 - rs3/2 - SC2*b3
        B1 = lambda t, h: bt[:, 2 * t + h: 2 * t + h + 1]
        B2 = lambda t, h: bt[:, 6 + 2 * t + h: 6 + 2 * t + h + 1]
        B3 = lambda t: bt[:, 12 + t: 13 + t]
        N1 = lambda t: bt[:, 15 + t: 16 + t]
        N2 = lambda t: bt[:, 18 + t: 19 + t]
        TH3C = bt[:, 21:22]

        # ---- persistent state (m = scaled membrane, written at t=0) ----
        m1 = [sp.tile([128, BC], bf16, tag=f"m1_{h}", name=f"m1_{h}")
              for h in range(2)]
        m2 = [sp.tile([128, BC], bf16, tag=f"m2_{h}", name=f"m2_{h}")
              for h in range(2)]
        m3 = sp.tile([128, BC], bf16, tag="m3")
        s1 = sp.tile([128, 2 * BC], f8, tag="s1")
        s2 = sp.tile([128, 2 * BC], f8, tag="s2")
        s1r = s1[:, :].rearrange("p (two n) -> p two n", two=2)
        s2r = s2[:, :].rearrange("p (two n) -> p two n", two=2)
        outsb = sp.tile([128, BC], f32, tag="outsb")

        xt = {}  # (t, blk) -> x tile handle [128, KD*2*NB]
        XCH = (0, 4 * 2 * NB, 8 * 2 * NB, KD * 2 * NB)  # k-chunks 0-3,4-7,8-10

        def x_alloc(t, blk):
            xt[t, blk] = xp.tile([128, KD * 2 * NB], f8, tag="x",
                                 name=f"x_{t}_{blk}")

        def x_fetch(t, blk, c, q=None):
            """Fetch k-chunk c of phase (t, blk)'s x slab (chunked so the
            k-loop's range deps release after ~1.5us, not the full slab)."""
            (q or nc.sync).dma_start(
                out=xt[t, blk][:, XCH[c]:XCH[c + 1]],
                in_=x[t, blk, :, XCH[c]:XCH[c + 1]])

        SSC = [2.0, 2.0, 4.0]  # L2/L3 extract scale: 2^t x (2 if +/-0.5 basis)

        def lif(ps, m_ap, s_ap, bcol, nthcol, th, t, P=128, sc=None):
            """Scaled-LIF on one [P, NB] psum span (t < T-1): ScalarE
            extract + bf16 VectorE ops; spike on ScalarE Sign at t=1,
            VectorE is_ge/sub at t=0."""
            hb = tp.tile([128, NB], bf16, tag="hb", name="hb")[:P, :]
            nc.scalar.activation(hb, ps, IDENT, bias=bcol[:P, :],
                                 scale=float(2 ** t) if sc is None else sc)
            if t == 0:
                u = hb
            else:
                u = tp.tile([128, NB], bf16, tag="u", name="u")[:P, :]
                nc.vector.tensor_tensor(u, m_ap, hb, A.add)
            if s_ap is not None:
                if t == 1:
                    nc.scalar.activation(s_ap, u, SIGN, bias=nthcol[:P, :])
                else:
                    nc.vector.tensor_scalar(s_ap, u, th * 2 ** t, 0.5,
                                            A.is_ge, A.subtract)
            nc.vector.scalar_tensor_tensor(m_ap, u, th * 2 ** t, u,
                                           A.is_lt, A.mult)

        def l1_lif(blk, ps):
            """L1 LIF at t=T-1 for one block, straight from PSUM:
            u = 0.25*m1 + psum; s1 = Sign(u - (TH1-EPS)) (+/-1 basis)."""
            cs = slice(blk * NB, (blk + 1) * NB)
            for h in range(2):
                u = tp.tile([128, NB], bf16, tag="u", name="u")
                nc.vector.scalar_tensor_tensor(u, m1[h][:, cs], 0.25,
                                               ps[:, h * NB:(h + 1) * NB],
                                               A.mult, A.add)
                nc.scalar.activation(s1[:, h * BC + blk * NB:
                                        h * BC + (blk + 1) * NB],
                                     u, SIGN, bias=N1(0))

        def l1_phase(t, blk, hooks=None):
            """One [2816 x 512] L1 GEMM phase: k inner, 2 MMs per slab into
            the two h-banks of one [128,1024] psum tile, then this
            (t, blk)'s L1 LIF. `hooks[k]` interleaves previous chains'
            matmuls and x prefetches into the PE stream."""
            cs = slice(blk * NB, (blk + 1) * NB)
            ps = pp1.tile([128, 2 * NB], f32, tag="ps1", name=f"ps_{t}_{blk}")
            xr = xt[t, blk][:, :].rearrange("p (k two n) -> p k two n",
                                            k=KD, two=2)
            for k in range(KD):
                for fn in (hooks or {}).get(k, []):
                    fn()
                for h in range(2):
                    nc.tensor.matmul(
                        ps[:, h * NB:(h + 1) * NB],
                        w1o[:, k, :, h * 128:(h + 1) * 128],
                        xr[:, k, :, :],
                        start=(k == 0), stop=(k == KD - 1), perf_mode=DR,
                        skip_group_check=True)
            if t < T - 1:
                for h in range(2):
                    lif(ps[:, h * NB:(h + 1) * NB], m1[h][:, cs],
                        s1[:, h * BC + blk * NB: h * BC + (blk + 1) * NB],
                        B1(t, h), N1(t), TH1, t)
            else:
                l1_lif(blk, ps)

        cht = {}  # (t, blk) -> shared L2/L3 chain psum tile

        def l2_blk(t, blk):
            """L2 chain for one (t, block): 2 MMs into one ps23 tile's
            h-banks, then per-h LIF (t<2) or the t=2 endgame extract."""
            cs = slice(blk * NB, (blk + 1) * NB)
            ch = pp23.tile([128, 2 * NB], f32, tag="ps23", name=f"ch_{t}_{blk}")
            cht[t, blk] = ch
            for h in range(2):
                nc.tensor.matmul(ch[:, h * NB:(h + 1) * NB],
                                 w2o[:, :, h * 128:(h + 1) * 128],
                                 s1r[:, :, blk * NB:(blk + 1) * NB],
                                 start=True, stop=True, perf_mode=DR,
                                 skip_group_check=True)
            for h in range(2):
                if t < T - 1:
                    lif(ch[:, h * NB:(h + 1) * NB], m2[h][:, cs],
                        s2[:, h * BC + blk * NB: h * BC + (blk + 1) * NB],
                        B2(t, h), N2(t), TH2, t, sc=SSC[t])
                else:
                    hb = tp.tile([128, NB], bf16, tag="hb", name="hb")
                    nc.scalar.activation(hb, ch[:, h * NB:(h + 1) * NB],
                                         IDENT, bias=B2(2, h), scale=SSC[2])
                    u = tp.tile([128, NB], bf16, tag="u", name="u")
                    nc.vector.tensor_tensor(u, m2[h][:, cs], hb, A.add)
                    nc.vector.tensor_scalar(
                        s2[:, h * BC + blk * NB: h * BC + (blk + 1) * NB],
                        u, TH2 * 4, 0.5, A.is_ge, A.subtract)

        def l3_blk(t, blk):
            """L3 chain for one (t, block), reusing the L2 chain tile's
            first bank. t<2 updates m3; t=2 compares u against the
            per-partition threshold column and DMAs the output block."""
            cs = slice(blk * NB, (blk + 1) * NB)
            ch = cht[t, blk]
            nc.tensor.matmul(ch[:OP, 0:NB], w3o[:, :, :],
                             s2r[:, :, blk * NB:(blk + 1) * NB],
                             start=True, stop=True, perf_mode=DR,
                             skip_group_check=True)
            if t < T - 1:
                lif(ch[:OP, 0:NB], m3[:OP, cs], None, B3(t), None, TH2, t,
                    P=OP, sc=SSC[t])
            else:
                u = tp.tile([128, NB], bf16, tag="u", name="u")[:OP, :]
                nc.vector.scalar_tensor_tensor(u, m3[:OP, cs], 0.125,
                                               ch[:OP, 0:NB], A.mult, A.add)
                nc.vector.tensor_scalar(outsb[:OP, cs], u, TH3C[:OP, :],
                                        None, A.is_ge)
                nc.scalar.dma_start(out=out[:, cs], in_=outsb[:O, cs])

        # ---- phase sequence and hook schedule ----
        phases = [(blk, t) for blk in range(NBLK) for t in range(T)]

        # Initial prefetch: phase 0's slab on sync (w1 k0 first -- it
        # gates the first matmul); w1's remaining slabs + phase 1's slab
        # + small tensors on scalar, in deadline order.
        nc.sync.dma_start(out=w1t[:, 0:512], in_=w1[:, 0:512])
        nc.scalar.dma_start(out=w1t[:, 512:2048], in_=w1[:, 512:2048])
        x_alloc(0, 0)
        x_fetch(0, 0, 0)
        nc.scalar.dma_start(out=w1t[:, 2048:4096], in_=w1[:, 2048:4096])
        x_fetch(0, 0, 1)
        nc.scalar.dma_start(out=w1t[:, 4096:KD * 512], in_=w1[:, 4096:KD * 512])
        x_fetch(0, 0, 2)
        x_alloc(1, 0)
        for c in range(3):
            x_fetch(1, 0, c, nc.scalar)
        nc.scalar.dma_start(out=bt[:, :], in_=bias[:, :])
        nc.scalar.dma_start(out=w2t[:, :], in_=w2[:, :])
        nc.scalar.dma_start(out=w3t[:, :], in_=w3[:, :])

        for pi, (blk, t) in enumerate(phases):
            hooks = {}
            # stream phase pi+2's x slab (3 chunks across this k-loop)
            if pi + 2 < len(phases):
                nb_, nt_ = phases[pi + 2]
                x_alloc(nt_, nb_)
                for c, kk in ((0, 0), (1, 4), (2, 8)):
                    hooks.setdefault(kk, []).append(
                        lambda nt_=nt_, nb_=nb_, c=c: x_fetch(nt_, nb_, c))
            # previous phase's L2 at k5, its L3 at k10 (L3 needs s2 from
            # the L2 chain, ~2us later)
            if pi >= 1:
                pb, pt = phases[pi - 1][0], phases[pi - 1][1]
                hooks.setdefault(5, []).append(
                    lambda pt=pt, pb=pb: l2_blk(pt, pb))
                hooks.setdefault(10, []).append(
                    lambda pt=pt, pb=pb: l3_blk(pt, pb))
            l1_phase(t, blk, hooks)
        # trailing chain of the last block's t=2
        l2_blk(2, NBLK - 1)
        l3_blk(2, NBLK - 1)

    nc.compile()
    return nc


def kernel(dvs, W1, b1, W2, b2, W3, b3):
    global _compiled, last_results
    from concourse.bass_utils import run_bass_kernel_spmd

    if _compiled is None:
        _compiled = _build()
    nc = _compiled

    f8 = ml_dtypes.float8_e4m3

    def q8(a, scale):
        return np.clip(a * scale, -240.0, 240.0).astype(f8)

    # x: [B, T, D] -> fp8 [T, KD, 128, 2, B]  (d = k*256 + two*128 + p)
    x8 = q8(dvs, 1.0).transpose(1, 2, 0)          # [T, D, B]
    X = np.zeros((T, KD, 2, 128, B), dtype=f8)
    X.reshape(T, DP, B)[:, :D, :] = x8
    X.reshape(T, DP, B)[:, D, :] = f8(1.0)        # bias row (w1 row D = c1*b1)
    X = np.ascontiguousarray(X.transpose(0, 1, 3, 2, 4))  # [T, KD, 128, 2, B]

    # w1: [DP, H] scaled by SC1 -> [128, KD, 2, H]
    w1p = np.zeros((KD, 2, 128, H), dtype=f8)
    w1p.reshape(DP, H)[:D, :] = q8(W1.T, SC1)
    w1p.reshape(DP, H)[D, :] = q8(b1, SC1)
    w1p = np.ascontiguousarray(w1p.transpose(2, 0, 1, 3)).reshape(128, KD * 2 * H)
    # w2/w3 scaled by SC2 (+/-1 spike basis)
    w2q = q8(W2.T, SC2)                            # [H, H] j-major
    w2p = np.ascontiguousarray(
        w2q.reshape(2, 128, H).transpose(1, 0, 2)).reshape(128, 2 * H)
    w3q = np.zeros((H, OP), dtype=f8)
    w3q[:, :O] = q8(W3.T, SC2)
    w3p = np.ascontiguousarray(
        w3q.reshape(2, 128, OP).transpose(1, 0, 2)).reshape(128, 2 * OP)

    # bias/threshold columns; row-sum corrections use the quantized weights
    rs2 = w2q.astype(np.float64).sum(axis=0)       # [H]
    rs3 = w3q.astype(np.float64).sum(axis=0)       # [OP]
    bc = np.zeros((128, 22), dtype=np.float32)
    for t in range(T):
        p2 = float(2 ** t)
        for h in range(2):
            bc[:, 6 + 2 * t + h] = p2 * (rs2[h * 128:(h + 1) * 128]
                                         + 2 * SC2 * b2[h * 128:(h + 1) * 128])
        bc[:OP, 12 + t] = p2 * rs3
        bc[:O, 12 + t] += p2 * 2 * SC2 * b3
        bc[:, 15 + t] = -(p2 * TH1 - EPS)
        bc[:, 18 + t] = -(p2 * TH2 - EPS)
    # final L3 threshold column: s3 = (0.125*m3 + psum >= 16 - corr),
    # corr = rs3/2 + SC2*b3 (the +/-0.5 s2 basis row-sum correction)
    bc[:OP, 21] = 16.0 - rs3 / 2
    bc[:O, 21] -= SC2 * b3

    in_maps = []
    for c in range(NCORES):
        xc = X[:, :, :, :, c * BC:(c + 1) * BC]    # [T, KD, 128, 2, BC]
        xc = np.ascontiguousarray(
            xc.reshape(T, KD, 128, 2, NBLK, NB).transpose(0, 4, 2, 1, 3, 5)
        ).reshape(T, NBLK, 128, KD * 2 * NB)       # [T, blk, 128, k-major]
        in_maps.append({"x": xc, "w1": w1p, "w2": w2p, "w3": w3p, "bias": bc})

    trace = bool(os.environ.get("SNN_TRACE"))
    last_results = run_bass_kernel_spmd(nc, in_maps, core_ids=list(range(NCORES)),
                                        trace=trace)
    outp = np.empty((B, O), dtype=np.float32)
    for c in range(NCORES):
        outp[c * BC:(c + 1) * BC, :] = last_results.results[c]["out"].T
    return outp


# revision 22
# speedup vs baseline: 1.2881x; 1.2451x over previous
"""Trainium2 Bass kernel for NeuroVPR Vanilla SNN (3-layer LIF, T=3).

Data-parallel over batch: B=16384 -> 2048 per core x 8 cores.

Math (per timestep, per layer): v = (v_prev + h)/2; s = (v>=1); v *= (1-s).
The LIF recurrence is homogeneous and the decay is a power of 2, so each
layer runs in a scaled basis u_t = 2^t * 2c * v_t (c = weight prescale,
32 for L1 / 16 for L2-L3, lifting weights out of fp8's subnormal range):
    u_t = m_{t-1} + 2^t * psum_t      (the *0.5 decay cancels)
    s_t = (u_t >= 2^t * 2c)
    m_t = u_t * (u_t < 2^t * 2c)
The 2^t factor rides the ScalarE extract's free `scale` field; thresholds
double each timestep (exact powers of 2). Spike decisions match the fp32
recurrence up to matmul quantization error.

All matmuls are fp8e4 perf_mode=DoubleRow (K=256/instr, warm issue rate
216 ns at N=512). Hidden-layer spikes live in a +/-1 (ScalarE Sign, t=1)
or +/-0.5 (VectorE is_ge/sub, t=0 and t=2) basis: the next layer's
ScalarE extract scale absorbs the basis change (SSC) and the row-sum
correction rides that layer's bias column, precomputed on host from the
quantized weights. L1's bias rides a ones row appended to x. L3 at t=2
needs no extract or correction matmul: u = 0.125*m3 + psum on VectorE
and the correction folds into a per-partition threshold column
(tensor_scalar with AP scalar).

Schedule is COLUMN-MAJOR: the batch splits into four 512-column blocks
and each block runs t=0,1,2 consecutively (columns are independent LIF
chains, so blocks are independent). Each phase = one L1 [2816x512] GEMM
(22 DR matmuls, ~4.75us); the previous phase's LIF + L2 + L3 chains run
on Scalar/Vector DURING the next phase, with their matmuls hooked into
its k-loop. This spreads the elementwise work evenly (~75% V / ~70% S
per phase) instead of stacking the whole t=2 dependency cone after the
last L1 matmul (the old layer-major tail was ~16us of serialized
VectorE; the HAM throttle also halves PE rate after ~68-75us of
sustained activity, so the tail must avoid PE). Only the last block's
short t=2 chain (~4us) trails the final matmul.

x tiles ([128, 2048] fp8 per (t, k-slab, half-batch)) stream in
consumption order ~one phase ahead, issues split across the sync and
scalar HW DGE queues (the only HW queues; gpsimd's software queue is
slow and its epilogue drains are expensive when used). w1's k=0 slab
rides the sync queue first: it gates the first matmul.
"""
import os
import numpy as np
import ml_dtypes

B, T, D = 16384, 3, 2752
DP = 2816          # D padded to 11*256
KD = DP // 256     # 11 DoubleRow contraction slabs
H, O = 256, 100
OP = 112           # O padded so the DoubleRow pair-stride is 16B-aligned
NCORES = 8
BC = B // NCORES   # 2048
NB = 512           # matmul free-dim block = column-block width
HB = BC // 2       # half-batch per x tile (1024)
NBLK = 4           # column blocks per core

SC1, SC2 = 32.0, 8.0    # weight prescale: L1; L2/L3 (+/-1 spike basis)
TH1, TH2 = 64.0, 32.0   # base thresholds (scaled x2 each timestep)
EPS = 0.0625            # tie-break so Sign(u - (th-EPS)) == +/-1 with s=1 at u==th

_compiled = None
last_results = None  # BassKernelResults of the most recent run (for profiling)


def _build():
    from contextlib import ExitStack
    import concourse.bass as bass
    import concourse.mybir as mybir
    import concourse.tile as tile
    from concourse import bacc

    f8 = mybir.dt.float8e4
    bf16 = mybir.dt.bfloat16
    f32 = mybir.dt.float32
    A = mybir.AluOpType
    DR = mybir.MatmulPerfMode.DoubleRow
    IDENT = mybir.ActivationFunctionType.Identity
    SIGN = mybir.ActivationFunctionType.Sign

    nc = bacc.Bacc("TRN2", target_bir_lowering=False, debug=False)
    # x packed per (t, column-block): one contiguous [128, KD*2*512] slab
    # (k-major per partition line) so each phase streams exactly the bytes
    # it consumes, in consumption order.
    x = nc.dram_tensor("x", [T, NBLK, 128, KD * 2 * NB], f8,
                       kind="ExternalInput").ap()
    w1 = nc.dram_tensor("w1", [128, KD * 2 * H], f8, kind="ExternalInput").ap()
    w2 = nc.dram_tensor("w2", [128, 2 * H], f8, kind="ExternalInput").ap()
    w3 = nc.dram_tensor("w3", [128, 2 * OP], f8, kind="ExternalInput").ap()
    bias = nc.dram_tensor("bias", [128, 22], f32, kind="ExternalInput").ap()
    out = nc.dram_tensor("out", [O, BC], f32, kind="ExternalOutput").ap()

    with tile.TileContext(nc) as tc, ExitStack() as ctx:
        wp = ctx.enter_context(tc.tile_pool(name="wp", bufs=1))
        xp = ctx.enter_context(tc.tile_pool(name="xp", bufs=4))
        pp1 = ctx.enter_context(tc.tile_pool(name="pp1", bufs=2, space="PSUM"))
        pp23 = ctx.enter_context(tc.tile_pool(name="pp23", bufs=2, space="PSUM"))
        sp = ctx.enter_context(tc.tile_pool(name="sp", bufs=1))
        tp = ctx.enter_context(tc.tile_pool(name="tp", bufs=6))

        # ---- ACT warmup first: fully host-data-independent ----
        wu = wp.tile([128, 8], bf16)
        wub = wp.tile([128, 1], f32)
        nc.vector.memset(wu[:, :], 0.0)
        nc.vector.memset(wub[:, :], 0.0)
        nc.scalar.activation(wu[:, 0:4], wu[:, 4:8], IDENT, bias=wub[:, 0:1])

        # ---- weights / bias tiles (DMAs issued in the initial-prefetch
        # interleave below, in deadline order against the early x tiles) ----
        w1t = wp.tile([128, KD * 2 * H], f8)
        w1o = w1t[:, :].rearrange("p (k two m) -> p k two m", k=KD, two=2)
        bt = wp.tile([128, 22], f32)
        w2t = wp.tile([128, 2 * H], f8)
        w2o = w2t[:, :].rearrange("p (two m) -> p two m", two=2)
        w3t = wp.tile([128, 2 * OP], f8)
        w3o = w3t[:, :].rearrange("p (two m) -> p two m", two=2)
        # column layout (host fills): 0-5 beta1[t,h]; 6-11 beta2[t,h];
        # 12-14 beta3[t]; 15-17 -(2^t*TH1-EPS); 18-20 -(2^t*TH2-EPS);
        # 21 final L3 threshold 16 - rs3/2 - SC2*b3
        B1 = lambda t, h: bt[:, 2 * t + h: 2 * t + h + 1]
        B2 = lambda t, h: bt[:, 6 + 2 * t + h: 6 + 2 * t + h + 1]
        B3 = lambda t: bt[:, 12 + t: 13 + t]
        N1 = lambda t: bt[:, 15 + t: 16 + t]
        N2 = lambda t: bt[:, 18 + t: 19 + t]
        TH3C = bt[:, 21:22]

        # ---- persistent state (m = scaled membrane, written at t=0) ----
        m1 = [sp.tile([128, BC], bf16, tag=f"m1_{h}", name=f"m1_{h}")
              for h in range(2)]
        m2 = [sp.tile([128, BC], bf16, tag=f"m2_{h}", name=f"m2_{h}")
              for h in range(2)]
        m3 = sp.tile([128, BC], bf16, tag="m3")
        s1 = sp.tile([128, 2 * BC], f8, tag="s1")
        s2 = sp.tile([128, 2 * BC], f8, tag="s2")
        s1r = s1[:, :].rearrange("p (two n) -> p two n", two=2)
        s2r = s2[:, :].rearrange("p (two n) -> p two n", two=2)
        outsb = sp.tile([128, BC], f32, tag="outsb")

        xt = {}  # (t, blk) -> x tile handle [128, KD*2*NB]
        XCH = (0, 4 * 2 * NB, 8 * 2 * NB, KD * 2 * NB)  # k-chunks 0-3,4-7,8-10

        def x_alloc(t, blk):
            xt[t, blk] = xp.tile([128, KD * 2 * NB], f8, tag="x",
                                 name=f"x_{t}_{blk}")

        def x_fetch(t, blk, c, q=None):
            """Fetch k-chunk c of phase (t, blk)'s x slab (chunked so the
            k-loop's range deps release after ~1.5us, not the full slab)."""
            (q or nc.sync).dma_start(
                out=xt[t, blk][:, XCH[c]:XCH[c + 1]],
                in_=x[t, blk, :, XCH[c]:XCH[c + 1]])

        SSC = [2.0, 2.0, 4.0]  # L2/L3 extract scale: 2^t x (2 if +/-0.5 basis)

        def lif(ps, m_ap, s_ap, bcol, nthcol, th, t, P=128, sc=None):
            """Scaled-LIF on one [P, NB] psum span (t < T-1): ScalarE
            extract + bf16 VectorE ops; spike on ScalarE Sign at t=1,
            VectorE is_ge/sub at t=0."""
            hb = tp.tile([128, NB], bf16, tag="hb", name="hb")[:P, :]
            nc.scalar.activation(hb, ps, IDENT, bias=bcol[:P, :],
                                 scale=float(2 ** t) if sc is None else sc)
            if t == 0:
                u = hb
            else:
                u = tp.tile([128, NB], bf16, tag="u", name="u")[:P, :]
                nc.vector.tensor_tensor(u, m_ap, hb, A.add)
            if s_ap is not None:
                if t == 1:
                    nc.scalar.activation(s_ap, u, SIGN, bias=nthcol[:P, :])
                else:
                    nc.vector.tensor_scalar(s_ap, u, th * 2 ** t, 0.5,
                                            A.is_ge, A.subtract)
            nc.vector.scalar_tensor_tensor(m_ap, u, th * 2 ** t, u,
                                           A.is_lt, A.mult)

        def l1_lif(blk, ps):
            """L1 LIF at t=T-1 for one block, straight from PSUM:
            u = 0.25*m1 + psum; s1 = Sign(u - (TH1-EPS)) (+/-1 basis)."""
            cs = slice(blk * NB, (blk + 1) * NB)
            for h in range(2):
                u = tp.tile([128, NB], bf16, tag="u", name="u")
                nc.vector.scalar_tensor_tensor(u, m1[h][:, cs], 0.25,
                                               ps[:, h * NB:(h + 1) * NB],
                                               A.mult, A.add)
                nc.scalar.activation(s1[:, h * BC + blk * NB:
                                        h * BC + (blk + 1) * NB],
                                     u, SIGN, bias=N1(0))

        def l1_phase(t, blk, hooks=None):
            """One [2816 x 512] L1 GEMM phase: k inner, 2 MMs per slab into
            the two h-banks of one [128,1024] psum tile, then this
            (t, blk)'s L1 LIF. `hooks[k]` interleaves previous chains'
            matmuls and x prefetches into the PE stream."""
            cs = slice(blk * NB, (blk + 1) * NB)
            ps = pp1.tile([128, 2 * NB], f32, tag="ps1", name=f"ps_{t}_{blk}")
            xr = xt[t, blk][:, :].rearrange("p (k two n) -> p k two n",
                                            k=KD, two=2)
            for k in range(KD):
                for fn in (hooks or {}).get(k, []):
                    fn()
                for h in range(2):
                    nc.tensor.matmul(
                        ps[:, h * NB:(h + 1) * NB],
                        w1o[:, k, :, h * 128:(h + 1) * 128],
                        xr[:, k, :, :],
                        start=(k == 0), stop=(k == KD - 1), perf_mode=DR,
                        skip_group_check=True)
            if t < T - 1:
                for h in range(2):
                    lif(ps[:, h * NB:(h + 1) * NB], m1[h][:, cs],
                        s1[:, h * BC + blk * NB: h * BC + (blk + 1) * NB],
                        B1(t, h), N1(t), TH1, t)
            else:
                l1_lif(blk, ps)

        cht = {}  # (t, blk) -> shared L2/L3 chain psum tile

        def l2_blk(t, blk):
            """L2 chain for one (t, block): 2 MMs into one ps23 tile's
            h-banks, then per-h LIF (t<2) or the t=2 endgame extract."""
            cs = slice(blk * NB, (blk + 1) * NB)
            ch = pp23.tile([128, 2 * NB], f32, tag="ps23", name=f"ch_{t}_{blk}")
            cht[t, blk] = ch
            for h in range(2):
                nc.tensor.matmul(ch[:, h * NB:(h + 1) * NB],
                                 w2o[:, :, h * 128:(h + 1) * 128],
                                 s1r[:, :, blk * NB:(blk + 1) * NB],
                                 start=True, stop=True, perf_mode=DR,
                                 skip_group_check=True)
            for h in range(2):
                if t < T - 1:
                    lif(ch[:, h * NB:(h + 1) * NB], m2[h][:, cs],
                        s2[:, h * BC + blk * NB: h * BC + (blk + 1) * NB],
                        B2(t, h), N2(t), TH2, t, sc=SSC[t])
                else:
                    hb = tp.tile([128, NB], bf16, tag="hb", name="hb")
                    nc.scalar.activation(hb, ch[:, h * NB:(h + 1) * NB],
                                         IDENT, bias=B2(2, h), scale=SSC[2])
                    u = tp.tile([128, NB], bf16, tag="u", name="u")
                    nc.vector.tensor_tensor(u, m2[h][:, cs], hb, A.add)
                    nc.vector.tensor_scalar(
                        s2[:, h * BC + blk * NB: h * BC + (blk + 1) * NB],
                        u, TH2 * 4, 0.5, A.is_ge, A.subtract)

        def l3_blk(t, blk):
            """L3 chain for one (t, block), reusing the L2 chain tile's
            first bank. t<2 updates m3; t=2 compares u against the
            per-partition threshold column and DMAs the output block."""
            cs = slice(blk * NB, (blk + 1) * NB)
            ch = cht[t, blk]
            nc.tensor.matmul(ch[:OP, 0:NB], w3o[:, :, :],
                             s2r[:, :, blk * NB:(blk + 1) * NB],
                             start=True, stop=True, perf_mode=DR,
                             skip_group_check=True)
            if t < T - 1:
                lif(ch[:OP, 0:NB], m3[:OP, cs], None, B3(t), None, TH2, t,
                    P=OP, sc=SSC[t])
            else:
                u = tp.tile([128, NB], bf16, tag="u", name="u")[:OP, :]
                nc.vector.scalar_tensor_tensor(u, m3[:OP, cs], 0.125,
                                               ch[:OP, 0:NB], A.mult, A.add)
                nc.vector.tensor_scalar(outsb[:OP, cs], u, TH3C[:OP, :],
                                        None, A.is_ge)
                nc.scalar.dma_start(out=out[:, cs], in_=outsb[:O, cs])

        # ---- phase sequence and hook schedule ----
        phases = [(blk, t) for blk in range(NBLK) for t in range(T)]

        # Initial prefetch: phase 0's slab on sync (w1 k0 first -- it
        # gates the first matmul); w1's remaining slabs + phase 1's slab
        # + small tensors on scalar, in deadline order.
        nc.sync.dma_start(out=w1t[:, 0:512], in_=w1[:, 0:512])
        nc.scalar.dma_start(out=w1t[:, 512:2048], in_=w1[:, 512:2048])
        x_alloc(0, 0)
        x_fetch(0, 0, 0)
        nc.scalar.dma_start(out=w1t[:, 2048:4096], in_=w1[:, 2048:4096])
        x_fetch(0, 0, 1)
        nc.scalar.dma_start(out=w1t[:, 4096:KD * 512], in_=w1[:, 4096:KD * 512])
        x_fetch(0, 0, 2)
        x_alloc(1, 0)
        for c in range(3):
            x_fetch(1, 0, c, nc.scalar)
        nc.scalar.dma_start(out=bt[:, :], in_=bias[:, :])
        nc.scalar.dma_start(out=w2t[:, :], in_=w2[:, :])
        nc.scalar.dma_start(out=w3t[:, :], in_=w3[:, :])

        for pi, (blk, t) in enumerate(phases):
            hooks = {}
            # stream phase pi+2's x slab (3 chunks across this k-loop)
            if pi + 2 < len(phases):
                nb_, nt_ = phases[pi + 2]
                x_alloc(nt_, nb_)
                for c, kk in ((0, 0), (1, 4), (2, 8)):
                    hooks.setdefault(kk, []).append(
                        lambda nt_=nt_, nb_=nb_, c=c: x_fetch(nt_, nb_, c))
            # phase p-1's L2 at k8 and p-2's L3 at k2: one full phase of
            # slack for the spike chains on the in-order V/S queues, so
            # these hooked matmuls never stall the PE stream
            if pi >= 2:
                pb2, pt2 = phases[pi - 2]
                hooks.setdefault(2, []).append(
                    lambda pt2=pt2, pb2=pb2: l3_blk(pt2, pb2))
            if pi >= 1:
                pb, pt = phases[pi - 1]
                hooks.setdefault(8, []).append(
                    lambda pt=pt, pb=pb: l2_blk(pt, pb))
            l1_phase(t, blk, hooks)
        # trailing chains of the last two phases
        l3_blk(*reversed(phases[-2]))
        l2_blk(*reversed(phases[-1]))
        l3_blk(*reversed(phases[-1]))

    nc.compile()
    return nc


def kernel(dvs, W1, b1, W2, b2, W3, b3):
    global _compiled, last_results
    from concourse.bass_utils import run_bass_kernel_spmd

    if _compiled is None:
        _compiled = _build()
    nc = _compiled

    f8 = ml_dtypes.float8_e4m3

    def q8(a, scale):
        return np.clip(a * scale, -240.0, 240.0).astype(f8)

    # x: [B, T, D] -> fp8 [T, KD, 128, 2, B]  (d = k*256 + two*128 + p)
    x8 = q8(dvs, 1.0).transpose(1, 2, 0)          # [T, D, B]
    X = np.zeros((T, KD, 2, 128, B), dtype=f8)
    X.reshape(T, DP, B)[:, :D, :] = x8
    X.reshape(T, DP, B)[:, D, :] = f8(1.0)        # bias row (w1 row D = c1*b1)
    X = np.ascontiguousarray(X.transpose(0, 1, 3, 2, 4))  # [T, KD, 128, 2, B]

    # w1: [DP, H] scaled by SC1 -> [128, KD, 2, H]
    w1p = np.zeros((KD, 2, 128, H), dtype=f8)
    w1p.reshape(DP, H)[:D, :] = q8(W1.T, SC1)
    w1p.reshape(DP, H)[D, :] = q8(b1, SC1)
    w1p = np.ascontiguousarray(w1p.transpose(2, 0, 1, 3)).reshape(128, KD * 2 * H)
    # w2/w3 scaled by SC2 (+/-1 spike basis)
    w2q = q8(W2.T, SC2)                            # [H, H] j-major
    w2p = np.ascontiguousarray(
        w2q.reshape(2, 128, H).transpose(1, 0, 2)).reshape(128, 2 * H)
    w3q = np.zeros((H, OP), dtype=f8)
    w3q[:, :O] = q8(W3.T, SC2)
    w3p = np.ascontiguousarray(
        w3q.reshape(2, 128, OP).transpose(1, 0, 2)).reshape(128, 2 * OP)

    # bias/threshold columns; row-sum corrections use the quantized weights
    rs2 = w2q.astype(np.float64).sum(axis=0)       # [H]
    rs3 = w3q.astype(np.float64).sum(axis=0)       # [OP]
    bc = np.zeros((128, 22), dtype=np.float32)
    for t in range(T):
        p2 = float(2 ** t)
        for h in range(2):
            bc[:, 6 + 2 * t + h] = p2 * (rs2[h * 128:(h + 1) * 128]
                                         + 2 * SC2 * b2[h * 128:(h + 1) * 128])
        bc[:OP, 12 + t] = p2 * rs3
        bc[:O, 12 + t] += p2 * 2 * SC2 * b3
        bc[:, 15 + t] = -(p2 * TH1 - EPS)
        bc[:, 18 + t] = -(p2 * TH2 - EPS)
    # final L3 threshold column: s3 = (0.125*m3 + psum >= 16 - corr),
    # corr = rs3/2 + SC2*b3 (the +/-0.5 s2 basis row-sum correction)
    bc[:OP, 21] = 16.0 - rs3 / 2
    bc[:O, 21] -= SC2 * b3

    in_maps = []
    for c in range(NCORES):
        xc = X[:, :, :, :, c * BC:(c + 1) * BC]    # [T, KD, 128, 2, BC]
        xc = np.ascontiguousarray(
            xc.reshape(T, KD, 128, 2, NBLK, NB).transpose(0, 4, 2, 1, 3, 5)
        ).reshape(T, NBLK, 128, KD * 2 * NB)       # [T, blk, 128, k-major]
        in_maps.append({"x": xc, "w1": w1p, "w2": w2p, "w3": w3p, "bias": bc})

    trace = bool(os.environ.get("SNN_TRACE"))
    last_results = run_bass_kernel_spmd(nc, in_maps, core_ids=list(range(NCORES)),
                                        trace=trace)
    outp = np.empty((B, O), dtype=np.float32)
    for c in range(NCORES):
        outp[c * BC:(c + 1) * BC, :] = last_results.results[c]["out"].T
    return outp
